# revision 16
# baseline (speedup 1.0000x reference)
"""Binarized linear kernel for Trainium2 (8 NeuronCores).

Problem: per-direction binary "match count" GEMM.
  input        (B=64, D=128, I=512)  bool
  weight_noise (D=128, O=512, I=512) bool
  bias_noise   (D=128, O=512)        float32
  out[b,d,o] = (#matches(input[b,d,:], weight_noise[d,:,:]) > bias_noise[d,o])

Math: with +/-1 encoding x~=2x-1, w~=2w-1:
  matches = (I + sum_i x~ w~) / 2, so
  out = (dotpm > 2*bias - I), where dotpm is a single +/-1 GEMM per direction.
Host pre-encodes +/-1 in fp8_e4m3 (exact).  dotpm is always an even integer,
so comparing against the odd integer t = 2*floor(thr/2)+1 is exactly
equivalent and fits fp16 losslessly (|t| <= 513 < 2048).

The threshold is folded INTO the GEMM: during the DMA lead-in (warm-up
phase) a K=2 fp16 matmul per pair  sel[2,128].T @ (-t)[2,512]  (start=True)
initializes psum rows 0:64 with -t_even and rows 64:128 with -t_odd.  The
fp8 chunk matmuls then accumulate dotpm, leaving psum = dotpm - t, an odd
integer that is never 0.  The epilogue is a plain sign test, alternating
banks between engines (a PSUM bank is single-ported; ACT+DVE must not
touch the same bank concurrently):
  DVE:  u8 = (psum > 0)          (even pairs)
  ACT:  u8 = relu(psum) cast u8  (odd pairs; nonzero odd -> nonzero u8)
and the host maps nonzero -> True.  All arithmetic is exact in fp32 PSUM.

Sharding: D across the 8 cores (16 directions each), fully independent.
Directions processed in PAIRS packed into the two 64-column halves of the
PE array via tile_position (2x PE throughput at M=64).

Pipeline (raw bacc, hand-rolled semaphores):
  SP:  thr DMA first, then 18 weight pieces (pair 0 at 128KB granularity
       for an early pipeline start, 256KB after), all unchained FIFO.
  ACT: x half DMAs, odd-pair Relu epilogue, all out slice DMAs.
  PE:  N=128 warm-up matmuls (HAM clock-gate) + the 8 threshold matmuls,
       then per pair: 2x4 accumulating fp8 matmuls (nothing else).
  DVE: even-pair psum > 0 -> uint8.
  POOL: two-phase cleanup -- input-side sems are drained while the
       epilogue still runs; only pe/cmp/out reset after the last out DMA.
"""

import numpy as np

import sys

for _p in ("/opt/trn_rl_repo",):
    if _p not in sys.path:
        sys.path.insert(0, _p)

B, D, O, I = 64, 128, 512, 512
NCORES = 8
DL = D // NCORES   # directions per core (16)
NP = DL // 2       # direction pairs per core (8)
KC = I // 128      # contraction chunks of 128 (4)
NB = 8             # PSUM banks of [128, 512]
N_WARM = 12        # N=512 warm-up matmuls before the threshold matmuls
WCOL = NP * KC * 2 * O  # total weight free columns per core (16384)

# Weight pieces: pairs 0 and 7 split per chunk (4 x 128KB each) -- pair 0 so
# the first matmuls gate on a small early DMA, pair 7 so the tail chase
# after the final semaphore is one chunk, not four.  Pairs 1-6 per
# chunk-pair (2 x 256KB each).  Column layout (piece-major, contiguous):
#   128KB piece (pair p, chunk c): (j, o)
#   256KB piece (pair p, half cc): (j, ci, o)
# PIECES[i] = (col_lo, col_hi); PIECE_OF[(p, c)] = piece index gating chunk c.
PIECES = []
PIECE_OF = {}
FINE = (0, NP - 1)  # pairs with chunk-granular 128KB pieces
for p in range(NP):
    base = p * 4096
    if p in FINE:
        for c in range(KC):
            PIECE_OF[(p, c)] = len(PIECES)
            PIECES.append((base + c * 1024, base + (c + 1) * 1024))
    else:
        for cc in range(2):
            PIECE_OF[(p, 2 * cc)] = len(PIECES)
            PIECE_OF[(p, 2 * cc + 1)] = len(PIECES)
            PIECES.append((base + cc * 2048, base + (cc + 1) * 2048))
NW = len(PIECES)  # 20


def _w_even_odd_cols(p, c):
    """(even_lo, odd_lo) column offsets of the [128, 512] rhs slices."""
    base = p * 4096
    if p in FINE:
        return base + c * 1024, base + c * 1024 + 512
    base += (c // 2) * 2048
    ci = c % 2
    return base + ci * 512, base + 1024 + ci * 512


_NC_CACHE = {}


def _build_bass():
    import concourse.mybir as mybir
    from concourse import bacc

    fp8 = mybir.dt.float8e4
    f16 = mybir.dt.float16
    u8 = mybir.dt.uint8
    f32 = mybir.dt.float32

    nc = bacc.Bacc("TRN2")
    # DRAM layouts (host pre-arranged; weight pieces contiguous per piece):
    #   xt  [128, (p c j b)]    : xt[k, p, c, j, b] = xs[b, d0+2p+j, c*128+k]
    #   wt  [128, WCOL]         : piece-major, see PIECES above
    #   thr [2, 128 + NP*O] f16 : cols 0:128 sel block (row0=[1]*64+[0]*64,
    #                             row1=[0]*64+[1]*64); col 128+p*O+o holds
    #                             -t for the even (row0) / odd (row1) dir
    #   out [128, (p o)] u8     : rows 0:64 even dir, 64:128 odd; host unscrambles
    xt_d = nc.dram_tensor("xt", [128, NP * KC * 2 * B], fp8, kind="ExternalInput")
    wt_d = nc.dram_tensor("wt", [128, WCOL], fp8, kind="ExternalInput")
    thr_d = nc.dram_tensor("thr", [2, 128 + NP * O], f16, kind="ExternalInput")
    out_d = nc.dram_tensor("out", [128, NP * O], u8, kind="ExternalOutput")

    from contextlib import ExitStack

    with ExitStack() as ctx:
        x_sb = ctx.enter_context(nc.sbuf_tensor("x_sb", [128, NP * KC * 2 * B], fp8))
        w_sb = ctx.enter_context(nc.sbuf_tensor("w_sb", [128, WCOL], fp8))
        thr_sb = ctx.enter_context(nc.sbuf_tensor("thr_sb", [2, 128 + NP * O], f16))
        out_sb = ctx.enter_context(nc.sbuf_tensor("out_sb", [128, NP * O], u8))
        warm = ctx.enter_context(nc.sbuf_tensor("warm", [128, 512], fp8))
        psum = ctx.enter_context(nc.psum_tensor([128, NB * O], f32))
        sem_thr = ctx.enter_context(nc.semaphore("sem_thr"))
        sem_x = [ctx.enter_context(nc.semaphore(f"sem_x{k}")) for k in range(2)]
        sem_w = [ctx.enter_context(nc.semaphore(f"sem_w{q}")) for q in range(NW)]
        sem_pe = ctx.enter_context(nc.semaphore("sem_pe"))
        sem_cmp = ctx.enter_context(nc.semaphore("sem_cmp"))
        sem_act = ctx.enter_context(nc.semaphore("sem_act"))
        sem_out = ctx.enter_context(nc.semaphore("sem_out"))
        block = ctx.enter_context(nc.Block())

        xv = x_sb[:, :].rearrange("k (p c j b) -> k p c j b", p=NP, c=KC, j=2)

        XH = NP * KC * 2 * B // 2

        @block.sync
        def _(sync):
            # thr first (tiny, gates the threshold matmuls), then all weight
            # pieces unchained -- the HWDGE ring drains them FIFO at full
            # rate; SBUF holds everything so no flow control is needed.
            sync.dma_start(thr_sb[:, :], thr_d[:, :]).then_inc(sem_thr, 16)
            for q, (lo, hi) in enumerate(PIECES):
                sync.dma_start(w_sb[:, lo:hi], wt_d[:, lo:hi]).then_inc(
                    sem_w[q], 16
                )

        @block.scalar
        def _(sc):
            # x halves early, then the epilogue for ODD pairs (Relu over the
            # full bank; psum is a nonzero odd integer, so u8(relu) is 0 iff
            # the compare is false).  ACT issues every pair's out slice DMA.
            sc.dma_start(x_sb[:, 0:XH], xt_d[:, 0:XH]).then_inc(sem_x[0], 16)
            sc.dma_start(x_sb[:, XH:], xt_d[:, XH:]).then_inc(sem_x[1], 16)
            # Out slices are batched into 3 DMAs (64KB transfers only run at
            # ~140GB/s and were degrading the weight-piece cadence): pairs
            # 0-3 mid-stream, 4-6 late, and 7 alone so the final transfer's
            # drain+receipt tail is minimal.
            for p in range(NP):
                lo = p * O
                if p % 2 == 1:
                    sc.wait_ge(sem_pe, p + 1)
                    sc.activation(
                        out=out_sb[:, lo : lo + O],
                        in_=psum[:, lo : lo + O],
                        func=mybir.ActivationFunctionType.Relu,
                    ).then_inc(sem_act, 1)
                if p == 3:
                    sc.wait_ge(sem_cmp, 2)  # DVE pairs 0,2 done
                    sc.wait_ge(sem_act, 2)  # relu 1,3 retired
                    sc.dma_start(
                        out_d[:, 0 : 4 * O], out_sb[:, 0 : 4 * O]
                    ).then_inc(sem_out, 16)
                elif p == 6:
                    sc.wait_ge(sem_cmp, 4)  # DVE pairs 4,6 done
                    sc.wait_ge(sem_act, 3)  # relu 5 retired
                    sc.dma_start(
                        out_d[:, 4 * O : 7 * O], out_sb[:, 4 * O : 7 * O]
                    ).then_inc(sem_out, 16)
                elif p == 7:
                    sc.wait_ge(sem_act, 4)  # relu 7 retired
                    sc.dma_start(
                        out_d[:, 7 * O : 8 * O], out_sb[:, 7 * O : 8 * O]
                    ).then_inc(sem_out, 16)

        @block.gpsimd
        def _(g):
            # Two-phase cleanup so the NEFF can re-execute: input-side DMA
            # sems (thr/x/w) are done once the PE has consumed every piece
            # (sem_pe == NP), so drain them while the epilogue still runs;
            # pe/cmp/out reset only after the last out DMA lands.
            in_sems = [sem_thr, *sem_x, *sem_w]
            tail_sems = [sem_pe, sem_cmp, sem_act, sem_out]
            nums = sorted(s.num for s in in_sems + tail_sems)
            lo, hi = nums[0], nums[-1]
            assert nums == list(range(lo, hi + 1)), nums
            in_nums = sorted(s.num for s in in_sems)
            tail_nums = sorted(s.num for s in tail_sems)
            assert in_nums[-1] + 1 == tail_nums[0]
            g.wait_ge(sem_pe, NP)
            g.dma_reset(range(in_nums[0], in_nums[-1] + 1))
            g.sem_clear(range(in_nums[0], in_nums[-1] + 1))
            g.wait_ge(sem_out, 3 * 16)
            g.dma_reset(range(tail_nums[0], tail_nums[-1] + 1))
            g.sem_clear(range(tail_nums[0], tail_nums[-1] + 1))

        @block.tensor
        def _(t):
            # Warm-up matmuls keep the PE HAM clock-gate busy through the
            # DMA lead-in; the 8 threshold matmuls continue the warm-up
            # while initializing every bank (rows 0:64 = -t_even, rows
            # 64:128 = -t_odd), so the DMA-paced steady state below runs
            # nothing but the chunk matmuls.
            for _ in range(N_WARM):
                t.matmul(
                    psum[0:B, (NB - 1) * O : NB * O],
                    warm[:, 0:B],
                    warm[:, 0:O],
                    start=True,
                    stop=True,
                )
            t.wait_ge(sem_thr, 16)
            for p in range(NP):
                t.matmul(
                    psum[:, p * O : (p + 1) * O],
                    thr_sb[0:2, 0:128],
                    thr_sb[0:2, 128 + p * O : 128 + (p + 1) * O],
                    start=True,
                    stop=False,
                )
            t.wait_ge(sem_x[0], 16)
            for p in range(NP):
                if p == NP // 2:
                    t.wait_ge(sem_x[1], 16)
                bank = psum[:, p * O : (p + 1) * O]
                mm = None
                seen = set()
                for c in range(KC):
                    q = PIECE_OF[(p, c)]
                    if q not in seen:
                        seen.add(q)
                        t.wait_ge(sem_w[q], 16)
                    elo, olo = _w_even_odd_cols(p, c)
                    # even direction -> array columns 0:64, psum rows 0:64
                    t.matmul(
                        bank[0:B, :],
                        xv[:, p, c, 0, :],
                        w_sb[:, elo : elo + O],
                        start=False,
                        stop=(c == KC - 1),
                        tile_position=(0, 0),
                    )
                    # odd direction -> array columns 64:128, psum rows 64:128
                    mm = t.matmul(
                        bank[B : 2 * B, :],
                        xv[:, p, c, 1, :],
                        w_sb[:, olo : olo + O],
                        start=False,
                        stop=(c == KC - 1),
                        tile_position=(0, 64),
                    )
                mm.then_inc(sem_pe, 1)

        @block.vector
        def _(v):
            # Even pairs: psum > 0 -> uint8 0/1 over the full bank.
            for p in range(0, NP, 2):
                v.wait_ge(sem_pe, p + 1)
                lo = p * O
                v.tensor_scalar(
                    out=out_sb[:, lo : lo + O],
                    in0=psum[:, lo : lo + O],
                    scalar1=0.0,
                    scalar2=None,
                    op0=mybir.AluOpType.is_gt,
                ).then_inc(sem_cmp, 1)

    nc.compile()
    return nc


def _get_nc():
    if "nc" not in _NC_CACHE:
        _NC_CACHE["nc"] = _build_bass()
    return _NC_CACHE["nc"]


def _prep_inputs(input, weight_noise, bias_noise):
    import ml_dtypes

    fp8 = ml_dtypes.float8_e4m3
    x = np.asarray(input).astype(np.int8)  # (B, D, I) in {0,1}
    w = np.asarray(weight_noise).astype(np.int8)  # (D, O, I)
    bias = np.asarray(bias_noise).astype(np.float32)  # (D, O)

    xs = (2 * x - 1).astype(fp8)  # +/-1
    ws = (2 * w - 1).astype(fp8)
    # dotpm is even; the odd integer t = 2*floor(thr/2)+1 compares identically.
    # dotpm is in [-I, I], so clipping to +/-(I+1) changes nothing and keeps
    # -t exactly representable in fp16 for any bias values.
    thr = np.float32(2.0) * bias - np.float32(I)
    thr = 2.0 * np.floor(thr.astype(np.float64) / 2.0) + 1.0
    negt = -np.clip(thr, -(I + 1), I + 1).astype(np.float16)  # (D, O)

    in_maps = []
    for cidx in range(NCORES):
        dsl = slice(cidx * DL, (cidx + 1) * DL)
        # xt[k, p, c, j, b] = xs[b, d0+2p+j, c*128+k]
        xt = xs[:, dsl, :].transpose(2, 1, 0)  # (I, DL, B)
        xt = xt.reshape(KC, 128, NP, 2, B)  # (c, k, p, j, b)
        xt = xt.transpose(1, 2, 0, 3, 4)  # (k, p, c, j, b)
        xt = np.ascontiguousarray(xt).reshape(128, NP * KC * 2 * B)
        # weights, piece-major per PIECES
        w6 = ws[dsl].transpose(0, 2, 1)  # (DL, I, O)
        w6 = w6.reshape(NP, 2, KC, 128, O)  # (p, j, c, k, o)
        cols = []
        for p in range(NP):
            if p in FINE:  # (k, j, o) per chunk
                for c in range(KC):
                    cols.append(
                        w6[p, :, c].transpose(1, 0, 2).reshape(128, 2 * O)
                    )
            else:  # (k, j, ci, o) per chunk-pair
                for cc in range(2):
                    blkw = w6[p, :, 2 * cc : 2 * cc + 2]  # (j, ci, k, o)
                    cols.append(
                        blkw.transpose(2, 0, 1, 3).reshape(128, 2 * 2 * O)
                    )
        wt = np.ascontiguousarray(np.concatenate(cols, axis=1))
        assert wt.shape == (128, WCOL)
        # thr row 0 = even dir of each pair, row 1 = odd dir; sel block first
        th = negt[dsl].reshape(NP, 2, O)
        thp = np.zeros((2, 128 + NP * O), dtype=np.float16)
        thp[0, 0:B] = 1.0
        thp[1, B:128] = 1.0
        thp[0, 128:] = th[:, 0, :].reshape(-1)
        thp[1, 128:] = th[:, 1, :].reshape(-1)
        in_maps.append({"xt": xt, "wt": wt, "thr": thp})
    return in_maps


def _patch_walrus_args():
    """Let every DGE DMA spread across all 16 DMA engines (default splits a
    DMA over ~4), so single transfers run at full aggregate bandwidth."""
    from concourse import bass_utils as bu

    if getattr(bu, "_dge_patched", False):
        return
    orig = bu.get_walrus_args

    def patched(*a, **k):
        return ["--min-num-dma-engines-for-dge=16", *orig(*a, **k)]

    bu.get_walrus_args = patched
    bu._dge_patched = True


def kernel(input, weight_noise, bias_noise):
    from concourse import bass_utils

    _patch_walrus_args()
    in_maps = _prep_inputs(input, weight_noise, bias_noise)
    nc = _get_nc()
    res = bass_utils.run_bass_kernel_spmd(nc, in_maps, core_ids=list(range(NCORES)))
    full = np.empty((B, D, O), dtype=bool)
    for cidx, r in enumerate(res.results):
        ro = np.asarray(r["out"]).reshape(128, NP, O)
        dsl = slice(cidx * DL, (cidx + 1) * DL)
        blk = full[:, dsl, :]
        blk[:, 0::2, :] = ro[0:B].astype(bool)
        blk[:, 1::2, :] = ro[B : 2 * B].astype(bool)
    return full
